# revision 4
# baseline (speedup 1.0000x reference)
"""3-layer GCN (GCNConv+BN+ReLU x2, GCNConv+log_softmax) on 8 trn2 NeuronCores.

Strategy: nodes are degree-sorted and dealt round-robin to 8 cores (balanced
shards). Per layer: each core computes h = act.T @ W for its own nodes (PE),
scales rows by dinv (ACT), writes its shard, AllGather -> full table in DRAM.
Aggregation: edges are packed into dst-aligned slots (slot partition == dst
lane); dma_gather fetches dinv[src]*h[src] rows for 128-edge blocks, identity
matmuls accumulate blocks into PSUM (segment-sum with zero-row padding), and
the epilogue applies dinv[dst], BN+ReLU (folded into one ACT op) after a PE
transpose back to feature-major for the next layer's matmul.  Gather indices
are int16 with a mid-table base so signed offsets cover all 50008 table rows.
"""
import numpy as np

N = 50000
E = 800000
D_IN = 128
D_H = 128
D_OUT = 40
D_OUT_PAD = 64
BN_EPS = 1e-5
NCORES = 8
SHARD = N // NCORES              # 6250
SHARD_ROWS = SHARD + 1           # + trailing zero row
TBL_ROWS = SHARD_ROWS * NCORES   # 50008
NTILES = (SHARD + 127) // 128    # 49
BASE = 32768                     # gather base row (signed int16 offsets)
ZID = (NCORES - 1) * SHARD_ROWS + SHARD  # 50007: a zero row in the hi range


def _preprocess(x, src, dst):
    deg = np.bincount(dst, minlength=N).astype(np.float64) + 1.0
    dinv = (1.0 / np.sqrt(deg)).astype(np.float32)
    order = np.argsort(deg, kind="stable")
    core_of = np.empty(N, np.int64)
    pos_of = np.empty(N, np.int64)
    core_of[order] = np.arange(N) % NCORES
    pos_of[order] = np.arange(N) // NCORES
    tid = core_of * SHARD_ROWS + pos_of          # node -> table row id

    es = np.concatenate([src, np.arange(N)])     # + self loops
    ed = np.concatenate([dst, np.arange(N)])
    ec = core_of[ed]
    ep = pos_of[ed]
    sid_all = tid[es]

    # per-core per-pos counts -> per-tile block counts (max across cores)
    tile_max = np.zeros((NCORES, NTILES), np.int64)
    per_core = []
    for c in range(NCORES):
        sel = ec == c
        pos = ep[sel]
        sid = sid_all[sel]
        o = np.argsort(pos, kind="stable")
        pos, sid = pos[o], sid[o]
        counts = np.bincount(pos, minlength=SHARD)
        cpad = np.zeros(NTILES * 128, np.int64)
        cpad[:SHARD] = counts
        tile_max[c] = cpad.reshape(NTILES, 128).max(1)
        per_core.append((pos, sid, counts))
    blocks = tile_max.max(0)                     # [NTILES]

    # slot arrays: per tile, blocks[t]*128 slots + 16 trailing pad idxs
    slot_off = np.zeros(NTILES, np.int64)
    call_cols = blocks * 8 + 1                   # int16 cols per call (16/col)
    s = 0
    for t in range(NTILES):
        slot_off[t] = s
        s += blocks[t] * 128
    S_slots = int(s)

    idx_wrapped = []
    for c in range(NCORES):
        pos, sid, counts = per_core[c]
        starts = np.concatenate([[0], np.cumsum(counts)[:-1]])
        r = np.arange(len(pos)) - np.repeat(starts, counts)
        tt = pos // 128
        jj = pos % 128
        flat = slot_off[tt] + r * 128 + jj
        slots = np.full(S_slots, ZID, np.int64)
        slots[flat] = sid
        # build wrapped int16 index tile, call by call (16 extra pad idxs each)
        cols = []
        for t in range(NTILES):
            seg = np.full(blocks[t] * 128 + 16, ZID, np.int64)
            seg[:blocks[t] * 128] = slots[slot_off[t]:slot_off[t] + blocks[t] * 128]
            w = (seg - BASE).astype(np.int16).reshape(-1, 16).T  # [16, cols]
            cols.append(w)
        w16 = np.concatenate(cols, axis=1)
        idx_wrapped.append(np.tile(w16, (8, 1)))  # replicate to 128 partitions

    # per-core dinv (node-on-partition per tile) and shard node order
    dinv_own = []
    shard_nodes = []
    for c in range(NCORES):
        nodes = order[c::NCORES]
        shard_nodes.append(nodes)
        dpad = np.zeros(NTILES * 128, np.float32)
        dpad[:SHARD] = dinv[nodes]
        dinv_own.append(dpad.reshape(NTILES, 128).T.copy())  # [128, NTILES]
    return blocks, call_cols, idx_wrapped, dinv_own, shard_nodes


def _build(blocks, call_cols):
    import concourse.bass as bass
    import concourse.tile as tile
    from concourse import bacc, mybir

    f32 = mybir.dt.float32
    nc = bacc.Bacc("TRN2", num_devices=NCORES, debug=False)
    SC = int(call_cols.sum())
    xT_in = nc.dram_tensor("xT", [128, SHARD], f32, kind="ExternalInput")
    idx_in = nc.dram_tensor("idx", [128, SC], mybir.dt.int16, kind="ExternalInput")
    dinv_in = nc.dram_tensor("dinvown", [128, NTILES], f32, kind="ExternalInput")
    W1_in = nc.dram_tensor("W1", [128, D_H], f32, kind="ExternalInput")
    W2_in = nc.dram_tensor("W2", [128, D_H], f32, kind="ExternalInput")
    W3_in = nc.dram_tensor("W3", [128, D_OUT_PAD], f32, kind="ExternalInput")
    sb1_in = nc.dram_tensor("sb1", [128, 2], f32, kind="ExternalInput")
    sb2_in = nc.dram_tensor("sb2", [128, 2], f32, kind="ExternalInput")
    b3_in = nc.dram_tensor("b3rep", [128, D_OUT_PAD], f32, kind="ExternalInput")
    id_in = nc.dram_tensor("ident", [128, 128], f32, kind="ExternalInput")
    y_out = nc.dram_tensor("y", [SHARD, D_OUT], f32, kind="ExternalOutput")

    with tile.TileContext(nc) as tc:
        with tc.tile_pool(name="cst", bufs=1) as cst, \
             tc.tile_pool(name="act", bufs=1) as actp, \
             tc.tile_pool(name="wrk", bufs=3) as wrk, \
             tc.tile_pool(name="gb", bufs=3) as gb, \
             tc.tile_pool(name="ps", bufs=2, space="PSUM") as ps, \
             tc.tile_pool(name="dram", bufs=1, space="DRAM") as dram:

            idx_sb = cst.tile([128, SC], mybir.dt.int16)
            nc.gpsimd.dma_start(idx_sb[:], idx_in[:, :])
            dinv_sb = cst.tile([128, NTILES], f32)
            nc.gpsimd.dma_start(dinv_sb[:], dinv_in[:, :])
            W1 = cst.tile([128, D_H], f32)
            nc.gpsimd.dma_start(W1[:], W1_in[:, :])
            W2 = cst.tile([128, D_H], f32)
            nc.gpsimd.dma_start(W2[:], W2_in[:, :])
            W3 = cst.tile([128, D_OUT_PAD], f32)
            nc.gpsimd.dma_start(W3[:], W3_in[:, :])
            sb1 = cst.tile([128, 2], f32)
            nc.gpsimd.dma_start(sb1[:], sb1_in[:, :])
            sb2 = cst.tile([128, 2], f32)
            nc.gpsimd.dma_start(sb2[:], sb2_in[:, :])
            b3r = cst.tile([128, D_OUT_PAD], f32)
            nc.gpsimd.dma_start(b3r[:], b3_in[:, :])
            ident = cst.tile([128, 128], f32)
            nc.gpsimd.dma_start(ident[:], id_in[:, :])
            zrow = cst.tile([128, 128], f32)
            nc.vector.memset(zrow[:], 0.0)

            actA = actp.tile([128, NTILES * 128], f32, tag="actA")
            actB = actp.tile([128, NTILES * 128], f32, tag="actB")
            nc.gpsimd.dma_start(actA[:, :SHARD], xT_in[:, :])

            shards = []
            tables = []
            for l, fo in ((0, D_H), (1, D_H), (2, D_OUT_PAD)):
                sh = dram.tile([SHARD_ROWS, fo], f32, tag=f"shard{l}")
                tb = dram.tile([TBL_ROWS, fo], f32, tag=f"table{l}",
                               addr_space="Shared")
                shards.append(sh)
                tables.append(tb)

            col16 = np.zeros(NTILES + 1, np.int64)
            col16[1:] = np.cumsum(call_cols)
            max_blk = int(blocks.max())

            for l in range(3):
                fo = D_H if l < 2 else D_OUT_PAD
                W = (W1, W2, W3)[l]
                act_in = (actA, actB, actA)[l]
                act_next = (actB, actA, None)[l]
                sh, tb = shards[l], tables[l]

                # phase A: h = act.T @ W per own tile, scaled by dinv[node]
                for t in range(NTILES):
                    pt = min(128, SHARD - t * 128)
                    ph = ps.tile([128, fo], f32, tag="ph")
                    nc.tensor.matmul(ph[:pt, :], lhsT=act_in[:, t * 128:t * 128 + pt],
                                     rhs=W[:], start=True, stop=True)
                    hsb = wrk.tile([128, fo], f32, tag="hsb")
                    nc.scalar.activation(hsb[:pt, :], ph[:pt, :],
                                         mybir.ActivationFunctionType.Copy,
                                         scale=dinv_sb[:pt, t:t + 1])
                    nc.gpsimd.dma_start(sh[t * 128:t * 128 + pt, :], hsb[:pt, :])
                nc.gpsimd.dma_start(sh[SHARD:SHARD + 1, :], zrow[0:1, :fo])

                # phase B: AllGather shard -> table
                nc.gpsimd.collective_compute(
                    "AllGather", mybir.AluOpType.bypass,
                    replica_groups=[list(range(NCORES))],
                    ins=[sh.opt()], outs=[tb.opt()])

                # phase C: gather + segment-sum + epilogue per dst tile
                for t in range(NTILES):
                    nb = int(blocks[t])
                    pt = min(128, SHARD - t * 128)
                    gt = gb.tile([128, max_blk + 1, fo], f32, tag="g")
                    nc.gpsimd.dma_gather(
                        out_ap=gt[:, :nb + 1, :],
                        in_ap=tb[BASE:, :],
                        idxs_ap=idx_sb[:, col16[t]:col16[t + 1]],
                        num_idxs=nb * 128 + 16,
                        num_idxs_reg=nb * 128 + 16,
                        elem_size=fo,
                        single_packet=False,
                    )
                    pa = ps.tile([128, fo], f32, tag="pa")
                    for b in range(nb):
                        nc.tensor.matmul(pa[:], lhsT=ident[:], rhs=gt[:, b, :],
                                         start=(b == 0), stop=(b == nb - 1))
                    if l < 2:
                        sbv = (sb1, sb2)[l]
                        zt = wrk.tile([128, 128], f32, tag="zt")
                        nc.scalar.activation(zt[:], pa[:],
                                             mybir.ActivationFunctionType.Copy,
                                             scale=dinv_sb[:, t:t + 1])
                        pT = ps.tile([128, 128], f32, tag="pT")
                        nc.tensor.transpose(pT[:], zt[:], ident[:])
                        nc.scalar.activation(act_next[:, t * 128:(t + 1) * 128], pT[:],
                                             mybir.ActivationFunctionType.Relu,
                                             bias=sbv[:, 1:2], scale=sbv[:, 0:1])
                    else:
                        zt = wrk.tile([128, D_OUT_PAD], f32, tag="zt3")
                        nc.scalar.activation(zt[:], pa[:],
                                             mybir.ActivationFunctionType.Copy,
                                             scale=dinv_sb[:, t:t + 1])
                        nc.vector.tensor_tensor(zt[:], zt[:], b3r[:],
                                                op=mybir.AluOpType.add)
                        mx = wrk.tile([128, 1], f32, tag="mx")
                        nc.vector.tensor_reduce(mx[:], zt[:, :D_OUT],
                                                axis=mybir.AxisListType.X,
                                                op=mybir.AluOpType.max)
                        nmx = wrk.tile([128, 1], f32, tag="nmx")
                        nc.vector.tensor_scalar_mul(nmx[:], mx[:], -1.0)
                        ex = wrk.tile([128, D_OUT], f32, tag="ex")
                        se = wrk.tile([128, 1], f32, tag="se")
                        nc.scalar.activation(ex[:], zt[:, :D_OUT],
                                             mybir.ActivationFunctionType.Exp,
                                             bias=nmx[:, 0:1], accum_out=se[:, 0:1])
                        lse = wrk.tile([128, 1], f32, tag="lse")
                        nc.scalar.activation(lse[:], se[:],
                                             mybir.ActivationFunctionType.Ln)
                        ot = wrk.tile([128, D_OUT], f32, tag="ot")
                        nc.vector.tensor_scalar(ot[:], zt[:, :D_OUT],
                                                scalar1=mx[:, 0:1],
                                                scalar2=lse[:, 0:1],
                                                op0=mybir.AluOpType.subtract,
                                                op1=mybir.AluOpType.subtract)
                        nc.gpsimd.dma_start(y_out[t * 128:t * 128 + pt, :], ot[:pt, :])
    nc.compile()
    return nc


def prepare(x, src, dst, W1, b1, W2, b2, W3, b3,
            g1, be1, m1, v1, g2, be2, m2, v2):
    x = np.asarray(x, np.float32)
    src = np.asarray(src, np.int64)
    dst = np.asarray(dst, np.int64)
    blocks, call_cols, idx_wrapped, dinv_own, shard_nodes = _preprocess(x, src, dst)
    nc = _build(blocks, call_cols)

    s1 = np.asarray(g1, np.float32) / np.sqrt(np.asarray(v1, np.float32) + BN_EPS)
    bias1 = np.asarray(b1, np.float32) * s1 + (np.asarray(be1, np.float32)
                                               - np.asarray(m1, np.float32) * s1)
    s2 = np.asarray(g2, np.float32) / np.sqrt(np.asarray(v2, np.float32) + BN_EPS)
    bias2 = np.asarray(b2, np.float32) * s2 + (np.asarray(be2, np.float32)
                                               - np.asarray(m2, np.float32) * s2)
    sb1 = np.stack([s1, bias1], 1).astype(np.float32)
    sb2 = np.stack([s2, bias2], 1).astype(np.float32)
    W3p = np.zeros((128, D_OUT_PAD), np.float32)
    W3p[:, :D_OUT] = np.asarray(W3, np.float32)
    b3p = np.zeros(D_OUT_PAD, np.float32)
    b3p[:D_OUT] = np.asarray(b3, np.float32)
    b3rep = np.tile(b3p[None, :], (128, 1))
    ident = np.eye(128, dtype=np.float32)

    in_maps = []
    for c in range(NCORES):
        in_maps.append({
            "xT": x[shard_nodes[c]].T.copy(),
            "idx": idx_wrapped[c],
            "dinvown": dinv_own[c],
            "W1": np.asarray(W1, np.float32), "W2": np.asarray(W2, np.float32),
            "W3": W3p, "sb1": sb1, "sb2": sb2, "b3rep": b3rep, "ident": ident,
        })
    return nc, in_maps, shard_nodes


def kernel(**inputs):
    from concourse.bass_utils import run_bass_kernel_spmd

    nc, in_maps, shard_nodes = prepare(**inputs)
    res = run_bass_kernel_spmd(nc, in_maps, core_ids=list(range(NCORES)))
    out = np.zeros((N, D_OUT), np.float32)
    for c in range(NCORES):
        out[shard_nodes[c]] = res.results[c]["y"]
    return out


# revision 5
# speedup vs baseline: 1.0364x; 1.0364x over previous
"""3-layer GCN (GCNConv+BN+ReLU x2, GCNConv+log_softmax) on 8 trn2 NeuronCores.

Strategy: nodes are degree-sorted and dealt round-robin to 8 cores (balanced
shards). Per layer: each core computes h = act.T @ W for its own nodes (PE),
scales rows by dinv (ACT), writes its shard, AllGather -> full table in DRAM.
Aggregation: edges are packed into dst-aligned slots (slot partition == dst
lane); dma_gather fetches dinv[src]*h[src] rows for 128-edge blocks, identity
matmuls accumulate blocks into PSUM (segment-sum with zero-row padding), and
the epilogue applies dinv[dst], BN+ReLU (folded into one ACT op) after a PE
transpose back to feature-major for the next layer's matmul.  Gather indices
are int16 with a mid-table base so signed offsets cover all 50008 table rows.
"""
import numpy as np

N = 50000
E = 800000
D_IN = 128
D_H = 128
D_OUT = 40
D_OUT_PAD = 64
BN_EPS = 1e-5
NCORES = 8
SHARD = N // NCORES              # 6250
SHARD_ROWS = SHARD + 1           # + trailing zero row
TBL_ROWS = SHARD_ROWS * NCORES   # 50008
NTILES = (SHARD + 127) // 128    # 49
BASE = 32768                     # gather base row (signed int16 offsets)
ZID = (NCORES - 1) * SHARD_ROWS + SHARD  # 50007: a zero row in the hi range


def _preprocess(x, src, dst):
    deg = np.bincount(dst, minlength=N).astype(np.float64) + 1.0
    dinv = (1.0 / np.sqrt(deg)).astype(np.float32)
    order = np.argsort(deg, kind="stable")
    core_of = np.empty(N, np.int64)
    pos_of = np.empty(N, np.int64)
    core_of[order] = np.arange(N) % NCORES
    pos_of[order] = np.arange(N) // NCORES
    tid = core_of * SHARD_ROWS + pos_of          # node -> table row id

    es = np.concatenate([src, np.arange(N)])     # + self loops
    ed = np.concatenate([dst, np.arange(N)])
    ec = core_of[ed]
    ep = pos_of[ed]
    sid_all = tid[es]

    # per-core per-pos counts -> per-tile block counts (max across cores)
    tile_max = np.zeros((NCORES, NTILES), np.int64)
    per_core = []
    for c in range(NCORES):
        sel = ec == c
        pos = ep[sel]
        sid = sid_all[sel]
        o = np.argsort(pos, kind="stable")
        pos, sid = pos[o], sid[o]
        counts = np.bincount(pos, minlength=SHARD)
        cpad = np.zeros(NTILES * 128, np.int64)
        cpad[:SHARD] = counts
        tile_max[c] = cpad.reshape(NTILES, 128).max(1)
        per_core.append((pos, sid, counts))
    blocks = tile_max.max(0)                     # [NTILES]

    # slot arrays: per tile, blocks[t]*128 slots + 16 trailing pad idxs
    slot_off = np.zeros(NTILES, np.int64)
    call_cols = blocks * 8 + 1                   # int16 cols per call (16/col)
    s = 0
    for t in range(NTILES):
        slot_off[t] = s
        s += blocks[t] * 128
    S_slots = int(s)

    idx_wrapped = []
    for c in range(NCORES):
        pos, sid, counts = per_core[c]
        starts = np.concatenate([[0], np.cumsum(counts)[:-1]])
        r = np.arange(len(pos)) - np.repeat(starts, counts)
        tt = pos // 128
        jj = pos % 128
        flat = slot_off[tt] + r * 128 + jj
        slots = np.full(S_slots, ZID, np.int64)
        slots[flat] = sid
        # build wrapped int16 index tile, call by call (16 extra pad idxs each)
        cols = []
        for t in range(NTILES):
            seg = np.full(blocks[t] * 128 + 16, ZID, np.int64)
            seg[:blocks[t] * 128] = slots[slot_off[t]:slot_off[t] + blocks[t] * 128]
            w = (seg - BASE).astype(np.int16).reshape(-1, 16).T  # [16, cols]
            cols.append(w)
        w16 = np.concatenate(cols, axis=1)
        idx_wrapped.append(np.tile(w16, (8, 1)))  # replicate to 128 partitions

    # per-core dinv (node-on-partition per tile) and shard node order
    dinv_own = []
    shard_nodes = []
    for c in range(NCORES):
        nodes = order[c::NCORES]
        shard_nodes.append(nodes)
        dpad = np.zeros(NTILES * 128, np.float32)
        dpad[:SHARD] = dinv[nodes]
        dinv_own.append(dpad.reshape(NTILES, 128).T.copy())  # [128, NTILES]
    return blocks, call_cols, idx_wrapped, dinv_own, shard_nodes


def _build(blocks, call_cols):
    import concourse.bass as bass
    import concourse.tile as tile
    from concourse import bacc, mybir

    f32 = mybir.dt.float32
    nc = bacc.Bacc("TRN2", num_devices=NCORES, debug=False)
    SC = int(call_cols.sum())
    xT_in = nc.dram_tensor("xT", [128, SHARD], f32, kind="ExternalInput")
    idx_in = nc.dram_tensor("idx", [128, SC], mybir.dt.int16, kind="ExternalInput")
    dinv_in = nc.dram_tensor("dinvown", [128, NTILES], f32, kind="ExternalInput")
    W1_in = nc.dram_tensor("W1", [128, D_H], f32, kind="ExternalInput")
    W2_in = nc.dram_tensor("W2", [128, D_H], f32, kind="ExternalInput")
    W3_in = nc.dram_tensor("W3", [128, D_OUT_PAD], f32, kind="ExternalInput")
    sb1_in = nc.dram_tensor("sb1", [128, 2], f32, kind="ExternalInput")
    sb2_in = nc.dram_tensor("sb2", [128, 2], f32, kind="ExternalInput")
    b3_in = nc.dram_tensor("b3rep", [128, D_OUT_PAD], f32, kind="ExternalInput")
    id_in = nc.dram_tensor("ident", [128, 128], f32, kind="ExternalInput")
    y_out = nc.dram_tensor("y", [SHARD, D_OUT], f32, kind="ExternalOutput")

    with tile.TileContext(nc) as tc:
        with tc.tile_pool(name="cst", bufs=1) as cst, \
             tc.tile_pool(name="act", bufs=1) as actp, \
             tc.tile_pool(name="wrk", bufs=3) as wrk, \
             tc.tile_pool(name="gb", bufs=3) as gb, \
             tc.tile_pool(name="ps", bufs=2, space="PSUM") as ps, \
             tc.tile_pool(name="dram", bufs=1, space="DRAM") as dram:

            idx_sb = cst.tile([128, SC], mybir.dt.int16)
            nc.gpsimd.dma_start(idx_sb[:], idx_in[:, :])
            dinv_sb = cst.tile([128, NTILES], f32)
            nc.gpsimd.dma_start(dinv_sb[:], dinv_in[:, :])
            W1 = cst.tile([128, D_H], f32)
            nc.gpsimd.dma_start(W1[:], W1_in[:, :])
            W2 = cst.tile([128, D_H], f32)
            nc.gpsimd.dma_start(W2[:], W2_in[:, :])
            W3 = cst.tile([128, D_OUT_PAD], f32)
            nc.gpsimd.dma_start(W3[:], W3_in[:, :])
            sb1 = cst.tile([128, 2], f32)
            nc.gpsimd.dma_start(sb1[:], sb1_in[:, :])
            sb2 = cst.tile([128, 2], f32)
            nc.gpsimd.dma_start(sb2[:], sb2_in[:, :])
            b3r = cst.tile([128, D_OUT_PAD], f32)
            nc.gpsimd.dma_start(b3r[:], b3_in[:, :])
            ident = cst.tile([128, 128], f32)
            nc.gpsimd.dma_start(ident[:], id_in[:, :])
            identb = cst.tile([128, 128], mybir.dt.bfloat16)
            nc.vector.tensor_copy(identb[:], ident[:])
            zrow = cst.tile([128, 128], f32)
            nc.vector.memset(zrow[:], 0.0)
            zrowb = cst.tile([128, 128], mybir.dt.bfloat16)
            nc.vector.memset(zrowb[:], 0.0)

            actA = actp.tile([128, NTILES * 128], f32, tag="actA")
            actB = actp.tile([128, NTILES * 128], f32, tag="actB")
            nc.gpsimd.dma_start(actA[:, :SHARD], xT_in[:, :])

            shards = []
            tables = []
            for l, fo in ((0, D_H), (1, D_H), (2, D_OUT_PAD)):
                tdt = mybir.dt.bfloat16 if l < 2 else f32
                sh = dram.tile([SHARD_ROWS, fo], tdt, tag=f"shard{l}")
                tb = dram.tile([TBL_ROWS, fo], tdt, tag=f"table{l}",
                               addr_space="Shared")
                shards.append(sh)
                tables.append(tb)

            col16 = np.zeros(NTILES + 1, np.int64)
            col16[1:] = np.cumsum(call_cols)
            max_blk = int(blocks.max())

            for l in range(3):
                fo = D_H if l < 2 else D_OUT_PAD
                tdt = mybir.dt.bfloat16 if l < 2 else f32
                idT = identb if l < 2 else ident
                zr = zrowb if l < 2 else zrow
                W = (W1, W2, W3)[l]
                act_in = (actA, actB, actA)[l]
                act_next = (actB, actA, None)[l]
                sh, tb = shards[l], tables[l]

                # phase A: h = act.T @ W per own tile, scaled by dinv[node]
                for t in range(NTILES):
                    pt = min(128, SHARD - t * 128)
                    ph = ps.tile([128, fo], f32, tag="ph")
                    nc.tensor.matmul(ph[:pt, :], lhsT=act_in[:, t * 128:t * 128 + pt],
                                     rhs=W[:], start=True, stop=True)
                    hsb = wrk.tile([128, fo], tdt, tag="hsb")
                    nc.scalar.activation(hsb[:pt, :], ph[:pt, :],
                                         mybir.ActivationFunctionType.Copy,
                                         scale=dinv_sb[:pt, t:t + 1])
                    nc.gpsimd.dma_start(sh[t * 128:t * 128 + pt, :], hsb[:pt, :])
                nc.gpsimd.dma_start(sh[SHARD:SHARD + 1, :], zr[0:1, :fo])

                # phase B: AllGather shard -> table
                nc.gpsimd.collective_compute(
                    "AllGather", mybir.AluOpType.bypass,
                    replica_groups=[list(range(NCORES))],
                    ins=[sh.opt()], outs=[tb.opt()])

                # phase C: gather + segment-sum + epilogue per dst tile
                for t in range(NTILES):
                    nb = int(blocks[t])
                    pt = min(128, SHARD - t * 128)
                    gt = gb.tile([128, max_blk + 1, fo], tdt, tag="g")
                    nc.gpsimd.dma_gather(
                        out_ap=gt[:, :nb + 1, :],
                        in_ap=tb[BASE:, :],
                        idxs_ap=idx_sb[:, col16[t]:col16[t + 1]],
                        num_idxs=nb * 128 + 16,
                        num_idxs_reg=nb * 128 + 16,
                        elem_size=fo,
                        single_packet=False,
                    )
                    pa = ps.tile([128, fo], f32, tag="pa")
                    for b in range(nb):
                        nc.tensor.matmul(pa[:], lhsT=idT[:], rhs=gt[:, b, :],
                                         start=(b == 0), stop=(b == nb - 1))
                    if l < 2:
                        sbv = (sb1, sb2)[l]
                        zt = wrk.tile([128, 128], f32, tag="zt")
                        nc.scalar.activation(zt[:], pa[:],
                                             mybir.ActivationFunctionType.Copy,
                                             scale=dinv_sb[:, t:t + 1])
                        pT = ps.tile([128, 128], f32, tag="pT")
                        nc.tensor.transpose(pT[:], zt[:], ident[:])
                        nc.scalar.activation(act_next[:, t * 128:(t + 1) * 128], pT[:],
                                             mybir.ActivationFunctionType.Relu,
                                             bias=sbv[:, 1:2], scale=sbv[:, 0:1])
                    else:
                        zt = wrk.tile([128, D_OUT_PAD], f32, tag="zt3")
                        nc.scalar.activation(zt[:], pa[:],
                                             mybir.ActivationFunctionType.Copy,
                                             scale=dinv_sb[:, t:t + 1])
                        nc.vector.tensor_tensor(zt[:], zt[:], b3r[:],
                                                op=mybir.AluOpType.add)
                        mx = wrk.tile([128, 1], f32, tag="mx")
                        nc.vector.tensor_reduce(mx[:], zt[:, :D_OUT],
                                                axis=mybir.AxisListType.X,
                                                op=mybir.AluOpType.max)
                        nmx = wrk.tile([128, 1], f32, tag="nmx")
                        nc.vector.tensor_scalar_mul(nmx[:], mx[:], -1.0)
                        ex = wrk.tile([128, D_OUT], f32, tag="ex")
                        se = wrk.tile([128, 1], f32, tag="se")
                        nc.scalar.activation(ex[:], zt[:, :D_OUT],
                                             mybir.ActivationFunctionType.Exp,
                                             bias=nmx[:, 0:1], accum_out=se[:, 0:1])
                        lse = wrk.tile([128, 1], f32, tag="lse")
                        nc.scalar.activation(lse[:], se[:],
                                             mybir.ActivationFunctionType.Ln)
                        ot = wrk.tile([128, D_OUT], f32, tag="ot")
                        nc.vector.tensor_scalar(ot[:], zt[:, :D_OUT],
                                                scalar1=mx[:, 0:1],
                                                scalar2=lse[:, 0:1],
                                                op0=mybir.AluOpType.subtract,
                                                op1=mybir.AluOpType.subtract)
                        nc.gpsimd.dma_start(y_out[t * 128:t * 128 + pt, :], ot[:pt, :])
    nc.compile()
    return nc


def prepare(x, src, dst, W1, b1, W2, b2, W3, b3,
            g1, be1, m1, v1, g2, be2, m2, v2):
    x = np.asarray(x, np.float32)
    src = np.asarray(src, np.int64)
    dst = np.asarray(dst, np.int64)
    blocks, call_cols, idx_wrapped, dinv_own, shard_nodes = _preprocess(x, src, dst)
    nc = _build(blocks, call_cols)

    s1 = np.asarray(g1, np.float32) / np.sqrt(np.asarray(v1, np.float32) + BN_EPS)
    bias1 = np.asarray(b1, np.float32) * s1 + (np.asarray(be1, np.float32)
                                               - np.asarray(m1, np.float32) * s1)
    s2 = np.asarray(g2, np.float32) / np.sqrt(np.asarray(v2, np.float32) + BN_EPS)
    bias2 = np.asarray(b2, np.float32) * s2 + (np.asarray(be2, np.float32)
                                               - np.asarray(m2, np.float32) * s2)
    sb1 = np.stack([s1, bias1], 1).astype(np.float32)
    sb2 = np.stack([s2, bias2], 1).astype(np.float32)
    W3p = np.zeros((128, D_OUT_PAD), np.float32)
    W3p[:, :D_OUT] = np.asarray(W3, np.float32)
    b3p = np.zeros(D_OUT_PAD, np.float32)
    b3p[:D_OUT] = np.asarray(b3, np.float32)
    b3rep = np.tile(b3p[None, :], (128, 1))
    ident = np.eye(128, dtype=np.float32)

    in_maps = []
    for c in range(NCORES):
        in_maps.append({
            "xT": x[shard_nodes[c]].T.copy(),
            "idx": idx_wrapped[c],
            "dinvown": dinv_own[c],
            "W1": np.asarray(W1, np.float32), "W2": np.asarray(W2, np.float32),
            "W3": W3p, "sb1": sb1, "sb2": sb2, "b3rep": b3rep, "ident": ident,
        })
    return nc, in_maps, shard_nodes


def kernel(**inputs):
    from concourse.bass_utils import run_bass_kernel_spmd

    nc, in_maps, shard_nodes = prepare(**inputs)
    res = run_bass_kernel_spmd(nc, in_maps, core_ids=list(range(NCORES)))
    out = np.zeros((N, D_OUT), np.float32)
    for c in range(NCORES):
        out[shard_nodes[c]] = res.results[c]["y"]
    return out


# revision 6
# speedup vs baseline: 1.7270x; 1.6664x over previous
"""3-layer GCN (GCNConv+BN+ReLU x2, GCNConv+log_softmax) on 8 trn2 NeuronCores.

Strategy: nodes are degree-sorted and dealt round-robin to 8 cores (balanced
shards). Per layer: each core computes h = act.T @ W for its own nodes (PE),
scales rows by dinv (ACT), writes its shard, AllGather -> full table in DRAM.
Aggregation: edges are packed into dst-aligned slots (slot partition == dst
lane); dma_gather fetches dinv[src]*h[src] rows for 128-edge blocks, identity
matmuls accumulate blocks into PSUM (segment-sum with zero-row padding), and
the epilogue applies dinv[dst], BN+ReLU (folded into one ACT op) after a PE
transpose back to feature-major for the next layer's matmul.  Gather indices
are int16 with a mid-table base so signed offsets cover all 50008 table rows.
"""
import numpy as np

N = 50000
E = 800000
D_IN = 128
D_H = 128
D_OUT = 40
D_OUT_PAD = 64
BN_EPS = 1e-5
NCORES = 8
SHARD = N // NCORES              # 6250
SHARD_ROWS = SHARD + 1           # + trailing zero row
TBL_ROWS = SHARD_ROWS * NCORES   # 50008
NTILES = (SHARD + 127) // 128    # 49
BASE = 32768                     # gather base row (signed int16 offsets)
ZID = (NCORES - 1) * SHARD_ROWS + SHARD  # 50007: a zero row in the hi range


def _preprocess(x, src, dst):
    deg = np.bincount(dst, minlength=N).astype(np.float64) + 1.0
    dinv = (1.0 / np.sqrt(deg)).astype(np.float32)
    order = np.argsort(deg, kind="stable")
    core_of = np.empty(N, np.int64)
    pos_of = np.empty(N, np.int64)
    core_of[order] = np.arange(N) % NCORES
    pos_of[order] = np.arange(N) // NCORES
    tid = core_of * SHARD_ROWS + pos_of          # node -> table row id

    es = np.concatenate([src, np.arange(N)])     # + self loops
    ed = np.concatenate([dst, np.arange(N)])
    ec = core_of[ed]
    ep = pos_of[ed]
    sid_all = tid[es]

    # per-core per-pos counts -> per-tile block counts (max across cores)
    tile_max = np.zeros((NCORES, NTILES), np.int64)
    per_core = []
    for c in range(NCORES):
        sel = ec == c
        pos = ep[sel]
        sid = sid_all[sel]
        o = np.argsort(pos, kind="stable")
        pos, sid = pos[o], sid[o]
        counts = np.bincount(pos, minlength=SHARD)
        cpad = np.zeros(NTILES * 128, np.int64)
        cpad[:SHARD] = counts
        tile_max[c] = cpad.reshape(NTILES, 128).max(1)
        per_core.append((pos, sid, counts))
    blocks = tile_max.max(0)                     # [NTILES]

    # slot arrays: per tile, blocks[t]*128 slots + 16 trailing pad idxs
    slot_off = np.zeros(NTILES, np.int64)
    call_cols = blocks * 8 + 1                   # int16 cols per call (16/col)
    s = 0
    for t in range(NTILES):
        slot_off[t] = s
        s += blocks[t] * 128
    S_slots = int(s)

    idx_wrapped = []
    for c in range(NCORES):
        pos, sid, counts = per_core[c]
        starts = np.concatenate([[0], np.cumsum(counts)[:-1]])
        r = np.arange(len(pos)) - np.repeat(starts, counts)
        tt = pos // 128
        jj = pos % 128
        flat = slot_off[tt] + r * 128 + jj
        slots = np.full(S_slots, ZID, np.int64)
        slots[flat] = sid
        # build wrapped int16 index tile, call by call (16 extra pad idxs each)
        cols = []
        for t in range(NTILES):
            seg = np.full(blocks[t] * 128 + 16, ZID, np.int64)
            seg[:blocks[t] * 128] = slots[slot_off[t]:slot_off[t] + blocks[t] * 128]
            w = (seg - BASE).astype(np.int16).reshape(-1, 16).T  # [16, cols]
            cols.append(w)
        w16 = np.concatenate(cols, axis=1)
        idx_wrapped.append(np.tile(w16, (8, 1)))  # replicate to 128 partitions

    # per-core dinv (node-on-partition per tile) and shard node order
    dinv_own = []
    shard_nodes = []
    for c in range(NCORES):
        nodes = order[c::NCORES]
        shard_nodes.append(nodes)
        dpad = np.zeros(NTILES * 128, np.float32)
        dpad[:SHARD] = dinv[nodes]
        dinv_own.append(dpad.reshape(NTILES, 128).T.copy())  # [128, NTILES]
    return blocks, call_cols, idx_wrapped, dinv_own, shard_nodes


def _build(blocks, call_cols):
    import concourse.bass as bass
    import concourse.tile as tile
    from concourse import bacc, mybir

    f32 = mybir.dt.float32
    nc = bacc.Bacc("TRN2", num_devices=NCORES, debug=False, num_swdge_queues=4)
    SC = int(call_cols.sum())
    xT_in = nc.dram_tensor("xT", [128, SHARD], f32, kind="ExternalInput")
    idx_in = nc.dram_tensor("idx", [128, SC], mybir.dt.int16, kind="ExternalInput")
    dinv_in = nc.dram_tensor("dinvown", [128, NTILES], f32, kind="ExternalInput")
    W1_in = nc.dram_tensor("W1", [128, D_H], f32, kind="ExternalInput")
    W2_in = nc.dram_tensor("W2", [128, D_H], f32, kind="ExternalInput")
    W3_in = nc.dram_tensor("W3", [128, D_OUT_PAD], f32, kind="ExternalInput")
    sb1_in = nc.dram_tensor("sb1", [128, 2], f32, kind="ExternalInput")
    sb2_in = nc.dram_tensor("sb2", [128, 2], f32, kind="ExternalInput")
    b3_in = nc.dram_tensor("b3rep", [128, D_OUT_PAD], f32, kind="ExternalInput")
    id_in = nc.dram_tensor("ident", [128, 128], f32, kind="ExternalInput")
    y_out = nc.dram_tensor("y", [SHARD, D_OUT], f32, kind="ExternalOutput")

    with tile.TileContext(nc) as tc:
        with tc.tile_pool(name="cst", bufs=1) as cst, \
             tc.tile_pool(name="act", bufs=1) as actp, \
             tc.tile_pool(name="wrk", bufs=3) as wrk, \
             tc.tile_pool(name="gb", bufs=3) as gb, \
             tc.tile_pool(name="ps", bufs=2, space="PSUM") as ps, \
             tc.tile_pool(name="dram", bufs=1, space="DRAM") as dram:

            idx_sb = cst.tile([128, SC], mybir.dt.int16)
            nc.gpsimd.dma_start(idx_sb[:], idx_in[:, :])
            dinv_sb = cst.tile([128, NTILES], f32)
            nc.gpsimd.dma_start(dinv_sb[:], dinv_in[:, :])
            W1 = cst.tile([128, D_H], f32)
            nc.gpsimd.dma_start(W1[:], W1_in[:, :])
            W2 = cst.tile([128, D_H], f32)
            nc.gpsimd.dma_start(W2[:], W2_in[:, :])
            W3 = cst.tile([128, D_OUT_PAD], f32)
            nc.gpsimd.dma_start(W3[:], W3_in[:, :])
            sb1 = cst.tile([128, 2], f32)
            nc.gpsimd.dma_start(sb1[:], sb1_in[:, :])
            sb2 = cst.tile([128, 2], f32)
            nc.gpsimd.dma_start(sb2[:], sb2_in[:, :])
            b3r = cst.tile([128, D_OUT_PAD], f32)
            nc.gpsimd.dma_start(b3r[:], b3_in[:, :])
            ident = cst.tile([128, 128], f32)
            nc.gpsimd.dma_start(ident[:], id_in[:, :])
            identb = cst.tile([128, 128], mybir.dt.bfloat16)
            nc.vector.tensor_copy(identb[:], ident[:])
            zrow = cst.tile([128, 128], f32)
            nc.vector.memset(zrow[:], 0.0)
            zrowb = cst.tile([128, 128], mybir.dt.bfloat16)
            nc.vector.memset(zrowb[:], 0.0)

            actA = actp.tile([128, NTILES * 128], f32, tag="actA")
            actB = actp.tile([128, NTILES * 128], f32, tag="actB")
            nc.gpsimd.dma_start(actA[:, :SHARD], xT_in[:, :])

            shards = []
            tables = []
            for l, fo in ((0, D_H), (1, D_H), (2, D_OUT_PAD)):
                tdt = mybir.dt.bfloat16 if l < 2 else f32
                sh = dram.tile([SHARD_ROWS, fo], tdt, tag=f"shard{l}")
                tb = dram.tile([TBL_ROWS, fo], tdt, tag=f"table{l}",
                               addr_space="Shared")
                shards.append(sh)
                tables.append(tb)

            col16 = np.zeros(NTILES + 1, np.int64)
            col16[1:] = np.cumsum(call_cols)
            max_blk = int(blocks.max())

            for l in range(3):
                fo = D_H if l < 2 else D_OUT_PAD
                tdt = mybir.dt.bfloat16 if l < 2 else f32
                idT = identb if l < 2 else ident
                zr = zrowb if l < 2 else zrow
                W = (W1, W2, W3)[l]
                act_in = (actA, actB, actA)[l]
                act_next = (actB, actA, None)[l]
                sh, tb = shards[l], tables[l]

                # phase A: h = act.T @ W per own tile, scaled by dinv[node]
                for t in range(NTILES):
                    pt = min(128, SHARD - t * 128)
                    ph = ps.tile([128, fo], f32, tag="ph")
                    nc.tensor.matmul(ph[:pt, :], lhsT=act_in[:, t * 128:t * 128 + pt],
                                     rhs=W[:], start=True, stop=True)
                    hsb = wrk.tile([128, fo], tdt, tag="hsb")
                    nc.scalar.activation(hsb[:pt, :], ph[:pt, :],
                                         mybir.ActivationFunctionType.Copy,
                                         scale=dinv_sb[:pt, t:t + 1])
                    nc.gpsimd.dma_start(sh[t * 128:t * 128 + pt, :], hsb[:pt, :])
                nc.gpsimd.dma_start(sh[SHARD:SHARD + 1, :], zr[0:1, :fo])

                # phase B: AllGather shard -> table
                nc.gpsimd.collective_compute(
                    "AllGather", mybir.AluOpType.bypass,
                    replica_groups=[list(range(NCORES))],
                    ins=[sh.opt()], outs=[tb.opt()])

                # phase C: gather + segment-sum + epilogue per dst tile
                for t in range(NTILES):
                    nb = int(blocks[t])
                    pt = min(128, SHARD - t * 128)
                    gt = gb.tile([128, max_blk + 1, fo], tdt, tag="g")
                    nc.gpsimd.dma_gather(
                        out_ap=gt[:, :nb + 1, :],
                        in_ap=tb[BASE:, :],
                        idxs_ap=idx_sb[:, col16[t]:col16[t + 1]],
                        num_idxs=nb * 128 + 16,
                        num_idxs_reg=nb * 128 + 16,
                        elem_size=fo,
                        single_packet=False,
                        queue_num=t % 4,
                    )
                    pa = ps.tile([128, fo], f32, tag="pa")
                    for b in range(nb):
                        nc.tensor.matmul(pa[:], lhsT=idT[:], rhs=gt[:, b, :],
                                         start=(b == 0), stop=(b == nb - 1))
                    if l < 2:
                        sbv = (sb1, sb2)[l]
                        zt = wrk.tile([128, 128], f32, tag="zt")
                        nc.scalar.activation(zt[:], pa[:],
                                             mybir.ActivationFunctionType.Copy,
                                             scale=dinv_sb[:, t:t + 1])
                        pT = ps.tile([128, 128], f32, tag="pT")
                        nc.tensor.transpose(pT[:], zt[:], ident[:])
                        nc.scalar.activation(act_next[:, t * 128:(t + 1) * 128], pT[:],
                                             mybir.ActivationFunctionType.Relu,
                                             bias=sbv[:, 1:2], scale=sbv[:, 0:1])
                    else:
                        zt = wrk.tile([128, D_OUT_PAD], f32, tag="zt3")
                        nc.scalar.activation(zt[:], pa[:],
                                             mybir.ActivationFunctionType.Copy,
                                             scale=dinv_sb[:, t:t + 1])
                        nc.vector.tensor_tensor(zt[:], zt[:], b3r[:],
                                                op=mybir.AluOpType.add)
                        mx = wrk.tile([128, 1], f32, tag="mx")
                        nc.vector.tensor_reduce(mx[:], zt[:, :D_OUT],
                                                axis=mybir.AxisListType.X,
                                                op=mybir.AluOpType.max)
                        nmx = wrk.tile([128, 1], f32, tag="nmx")
                        nc.vector.tensor_scalar_mul(nmx[:], mx[:], -1.0)
                        ex = wrk.tile([128, D_OUT], f32, tag="ex")
                        se = wrk.tile([128, 1], f32, tag="se")
                        nc.scalar.activation(ex[:], zt[:, :D_OUT],
                                             mybir.ActivationFunctionType.Exp,
                                             bias=nmx[:, 0:1], accum_out=se[:, 0:1])
                        lse = wrk.tile([128, 1], f32, tag="lse")
                        nc.scalar.activation(lse[:], se[:],
                                             mybir.ActivationFunctionType.Ln)
                        ot = wrk.tile([128, D_OUT], f32, tag="ot")
                        nc.vector.tensor_scalar(ot[:], zt[:, :D_OUT],
                                                scalar1=mx[:, 0:1],
                                                scalar2=lse[:, 0:1],
                                                op0=mybir.AluOpType.subtract,
                                                op1=mybir.AluOpType.subtract)
                        nc.gpsimd.dma_start(y_out[t * 128:t * 128 + pt, :], ot[:pt, :])
    nc.compile()
    return nc


def prepare(x, src, dst, W1, b1, W2, b2, W3, b3,
            g1, be1, m1, v1, g2, be2, m2, v2):
    x = np.asarray(x, np.float32)
    src = np.asarray(src, np.int64)
    dst = np.asarray(dst, np.int64)
    blocks, call_cols, idx_wrapped, dinv_own, shard_nodes = _preprocess(x, src, dst)
    nc = _build(blocks, call_cols)

    s1 = np.asarray(g1, np.float32) / np.sqrt(np.asarray(v1, np.float32) + BN_EPS)
    bias1 = np.asarray(b1, np.float32) * s1 + (np.asarray(be1, np.float32)
                                               - np.asarray(m1, np.float32) * s1)
    s2 = np.asarray(g2, np.float32) / np.sqrt(np.asarray(v2, np.float32) + BN_EPS)
    bias2 = np.asarray(b2, np.float32) * s2 + (np.asarray(be2, np.float32)
                                               - np.asarray(m2, np.float32) * s2)
    sb1 = np.stack([s1, bias1], 1).astype(np.float32)
    sb2 = np.stack([s2, bias2], 1).astype(np.float32)
    W3p = np.zeros((128, D_OUT_PAD), np.float32)
    W3p[:, :D_OUT] = np.asarray(W3, np.float32)
    b3p = np.zeros(D_OUT_PAD, np.float32)
    b3p[:D_OUT] = np.asarray(b3, np.float32)
    b3rep = np.tile(b3p[None, :], (128, 1))
    ident = np.eye(128, dtype=np.float32)

    in_maps = []
    for c in range(NCORES):
        in_maps.append({
            "xT": x[shard_nodes[c]].T.copy(),
            "idx": idx_wrapped[c],
            "dinvown": dinv_own[c],
            "W1": np.asarray(W1, np.float32), "W2": np.asarray(W2, np.float32),
            "W3": W3p, "sb1": sb1, "sb2": sb2, "b3rep": b3rep, "ident": ident,
        })
    return nc, in_maps, shard_nodes


def kernel(**inputs):
    from concourse.bass_utils import run_bass_kernel_spmd

    nc, in_maps, shard_nodes = prepare(**inputs)
    res = run_bass_kernel_spmd(nc, in_maps, core_ids=list(range(NCORES)))
    out = np.zeros((N, D_OUT), np.float32)
    for c in range(NCORES):
        out[shard_nodes[c]] = res.results[c]["y"]
    return out
